# revision 19
# baseline (speedup 1.0000x reference)
"""Batched CNN-CRF Viterbi decode on 8 Trainium2 NeuronCores.

Problem: feats [4096, 700, 10] f32, lengths [4096] i32, transitions [10,10] f32.
Returns (path_scores [4096] f32, best_paths [4096, 700] i32) exactly matching
the jax reference (sequential Viterbi scan + backtrace, masked past lengths).

Sharding: pure data parallel. Batch 4096 -> 8 cores x 512 sequences.
Per-core layout: 512 = 4 groups x 128 partitions; tags live in the free dim.

Per forward step t (all 512 sequences in one instruction set, FD=(g,next,prev)=400):
  ntv  = fv[p,g,1,prev] + trans[p,1,next,prev]          (DVE tensor_tensor add)
  vv   = reduce_max(ntv, axis=prev)                     (DVE tensor_reduce)
  eq   = ntv >= vv                                      (DVE is_ge)
  sel  = eq * (9 - prev)                                (DVE mult w/ reversed iota)
  bprv = reduce_max(sel, axis=prev)   -> 9 - argmax_first(ntv)   (DVE)
  bp[t]= int8(9 - bprv)  (true argmax, first-match ties like jnp.argmax)  (ACT)
  fvn  = vv + feat[t]                                   (DVE)
  fv   = where(t < len, fvn, fv)                        (DVE copy_predicated)
Backpointers stay in SBUF (int8, 28KB/partition); backtrace runs on-chip via a
one-hot multiply + segmented sum gather; frozen lanes handled with the same
mask so bp past length never has to be identity.
"""

import os
import sys
import numpy as np

sys.path.insert(0, "/opt/trn_rl_repo")
sys.path.insert(0, "/root/.axon_site/_ro/pypackages")

P = 128          # SBUF partitions
G = 4            # batch groups per core (G*P = 512 sequences/core)
K = 10           # tagset size
NCORES = 8
START_TAG = 8
STOP_TAG = 9
NEG = -10000.0

_BUILD_CACHE = {}


def build_nc(T, Tc, bp_dtype="int8", debug_outs=False, lg=None):
    """Build the per-core Bass program (SPMD: same program on all cores).

    lg: per-group static max lengths (descending). Batch groups are
    length-sorted on the host, so group g needs only lg[g] time steps; each
    per-step instruction covers only the still-active group prefix.
    """
    import concourse.mybir as mybir
    from concourse import bacc
    import concourse.tile as tile

    f32 = mybir.dt.float32
    i32 = mybir.dt.int32
    i8 = getattr(mybir.dt, bp_dtype)
    Alu = mybir.AluOpType
    X = mybir.AxisListType.X

    if lg is None:
        lg = (T,) * G
    assert len(lg) == G and all(lg[i] >= lg[i + 1] for i in range(G - 1))
    Lmax = int(lg[0])

    def n_active(t):
        return sum(1 for x in lg if x > t)

    assert T % Tc == 0

    nc = bacc.Bacc(None, target_bir_lowering=False)

    feats_d = nc.dram_tensor("feats", (P, T, G, K), f32, kind="ExternalInput")
    lens_d = nc.dram_tensor("lens", (P, G), f32, kind="ExternalInput")
    consts_d = nc.dram_tensor("consts", (P, 128), f32, kind="ExternalInput")
    scores_d = nc.dram_tensor("scores", (P, G), f32, kind="ExternalOutput")
    path_d = nc.dram_tensor("path", (P, T, G), i32, kind="ExternalOutput")
    if debug_outs:
        bp_dbg = nc.dram_tensor("bp_dbg", (P, T, G, K), f32, kind="ExternalOutput")
        ph_dbg = nc.dram_tensor("ph_dbg", (P, T, G), f32, kind="ExternalOutput")

    with tile.TileContext(nc) as tc:
        with (
            tc.tile_pool(name="persist", bufs=1) as pp,
            tc.tile_pool(name="fstream", bufs=2) as fp,
            tc.tile_pool(name="work", bufs=2) as wp,
        ):
            consts = pp.tile([P, 128], f32, tag="consts")
            nc.sync.dma_start(consts[:], consts_d[:])
            lens = pp.tile([P, G], f32, tag="lens")
            nc.sync.dma_start(lens[:], lens_d[:])

            fv = pp.tile([P, G, K], f32, tag="fv")
            bp = pp.tile([P, T, G, K], i8, tag="bp")
            ph = pp.tile([P, T, G], f32, tag="ph")
            mask = pp.tile([P, T, G], f32, tag="mask")
            maskp = pp.tile([P, T, G], mybir.dt.int8, tag="maskp")
            maskx = pp.tile([P, T, G, K], mybir.dt.int8, tag="maskx")
            iota_t = pp.tile([P, T], i32, tag="iota_t")
            iota_f = pp.tile([P, T], f32, tag="iota_f")
            tagc = pp.tile([P, G], f32, tag="tagc")
            sc = pp.tile([P, G], f32, tag="sc")

            # fv0: NEG everywhere, 0 at START_TAG
            nc.vector.memset(fv[:], NEG)
            nc.vector.memset(fv[:, :, START_TAG : START_TAG + 1], 0.0)
            # ph regions past each group's maxlen are never written by the
            # backtrace; zero once so the final mask-multiply can't hit NaNs.
            nc.vector.memset(ph[:], 0.0)

            # mask[p, t, g] = (t < len[p, g])
            nc.gpsimd.iota(iota_t[:], pattern=[[1, T]], base=0, channel_multiplier=0)
            nc.vector.tensor_copy(iota_f[:], iota_t[:])
            for g in range(G):
                nc.vector.tensor_scalar(
                    mask[:, :, g],
                    iota_f[:],
                    lens[:, g : g + 1],
                    None,
                    op0=Alu.is_lt,
                )
                nc.vector.tensor_scalar(
                    maskp[:, :, g],
                    iota_f[:],
                    lens[:, g : g + 1],
                    None,
                    op0=Alu.is_lt,
                )
                # expanded (t, g, k) int8 mask for per-step copy_predicated
                nc.vector.tensor_scalar(
                    maskx[:, :, g, :],
                    iota_f[:].unsqueeze(2).broadcast_to((P, T, K)),
                    lens[:, g : g + 1],
                    None,
                    op0=Alu.is_lt,
                )

            # broadcast views of the constant rows, per active-group count
            trans_b = {
                ga: consts[:, 0:100]
                .rearrange("p (n k) -> p n k", n=K)
                .unsqueeze(1)
                .broadcast_to((P, ga, K, K))
                for ga in range(1, G + 1)
            }
            rev_b4 = {
                ga: consts[:, 100:110]
                .unsqueeze(1)
                .unsqueeze(1)
                .broadcast_to((P, ga, K, K))
                for ga in range(1, G + 1)
            }  # (9 - prev) along innermost
            rev_b3 = consts[:, 100:110].unsqueeze(1).broadcast_to((P, G, K))
            iota_b3 = {
                ga: consts[:, 110:120].unsqueeze(1).broadcast_to((P, ga, K))
                for ga in range(1, G + 1)
            }
            stop_b = (
                consts[:, 90:100].unsqueeze(1).broadcast_to((P, G, K))
            )  # trans[STOP, :]

            # ---------------- forward scan ----------------
            nchunks_f = (Lmax + Tc - 1) // Tc
            for c in range(nchunks_f):
                t0c = c * Tc
                tc_len = min(Tc, Lmax - t0c)
                ga_c = n_active(t0c)
                ft = fp.tile([P, Tc, G, K], f32, tag="ft")
                nc.sync.dma_start(
                    ft[:, 0:tc_len, 0:ga_c, :],
                    feats_d[:, t0c : t0c + tc_len, 0:ga_c, :],
                )
                for tl in range(tc_len):
                    t = t0c + tl
                    ga = n_active(t)
                    ntv = wp.tile([P, G, K, K], f32, tag="ntv")
                    fv_b = fv[:, 0:ga].unsqueeze(2).broadcast_to((P, ga, K, K))
                    nc.vector.tensor_tensor(
                        ntv[:, 0:ga], fv_b, trans_b[ga], op=Alu.add
                    )

                    vv = wp.tile([P, G, K], f32, tag="vv")
                    nc.vector.tensor_reduce(
                        vv[:, 0:ga], ntv[:, 0:ga], axis=X, op=Alu.max
                    )

                    eq = wp.tile([P, G, K, K], f32, tag="eq")
                    vv_b = vv[:, 0:ga].unsqueeze(3).broadcast_to((P, ga, K, K))
                    nc.vector.tensor_tensor(
                        eq[:, 0:ga], ntv[:, 0:ga], vv_b, op=Alu.is_ge
                    )

                    sel = wp.tile([P, G, K, K], f32, tag="sel")
                    nc.vector.tensor_tensor(
                        sel[:, 0:ga], eq[:, 0:ga], rev_b4[ga], op=Alu.mult
                    )

                    bpr = wp.tile([P, G, K], f32, tag="bpr")
                    nc.vector.tensor_reduce(
                        bpr[:, 0:ga], sel[:, 0:ga], axis=X, op=Alu.max
                    )

                    # bp[t] = 9 - bpr (true first-match argmax), cast to int8
                    nc.scalar.activation(
                        bp[:, t, 0:ga, :],
                        bpr[:, 0:ga],
                        mybir.ActivationFunctionType.Copy,
                        bias=9.0,
                        scale=-1.0,
                    )

                    fvn = wp.tile([P, G, K], f32, tag="fvn")
                    nc.vector.tensor_tensor(
                        fvn[:, 0:ga], vv[:, 0:ga], ft[:, tl, 0:ga, :], op=Alu.add
                    )
                    nc.vector.copy_predicated(
                        fv[:, 0:ga], maskx[:, t, 0:ga, :], fvn[:, 0:ga]
                    )

            # ---------------- terminal ----------------
            term = wp.tile([P, G, K], f32, tag="fvn")
            nc.vector.tensor_tensor(term[:], fv[:], stop_b, op=Alu.add)
            nc.vector.tensor_reduce(sc[:], term[:], axis=X, op=Alu.max)
            nc.sync.dma_start(scores_d[:], sc[:])

            eqt = wp.tile([P, G, K], f32, tag="vv")
            sc_b = sc[:].unsqueeze(2).broadcast_to((P, G, K))
            nc.vector.tensor_tensor(eqt[:], term[:], sc_b, op=Alu.is_ge)
            selt = wp.tile([P, G, K], f32, tag="bpr")
            nc.vector.tensor_tensor(selt[:], eqt[:], rev_b3, op=Alu.mult)
            ltr = wp.tile([P, G], f32, tag="ltr")
            nc.vector.tensor_reduce(ltr[:], selt[:], axis=X, op=Alu.max)
            nc.scalar.activation(
                tagc[:],
                ltr[:],
                mybir.ActivationFunctionType.Copy,
                bias=9.0,
                scale=-1.0,
            )

            # ---------------- backtrace ----------------
            for t in reversed(range(Lmax)):
                ga = n_active(t)
                nc.vector.tensor_copy(ph[:, t, 0:ga], tagc[:, 0:ga])
                oh = wp.tile([P, G, K], f32, tag="oh")
                tag_b = tagc[:, 0:ga].unsqueeze(2).broadcast_to((P, ga, K))
                nc.vector.tensor_tensor(
                    oh[:, 0:ga], iota_b3[ga], tag_b, op=Alu.is_equal
                )
                gat = wp.tile([P, G, K], f32, tag="gat")
                nc.vector.tensor_tensor(
                    gat[:, 0:ga], oh[:, 0:ga], bp[:, t, 0:ga, :], op=Alu.mult
                )
                tn = wp.tile([P, G], f32, tag="tn")
                nc.vector.tensor_reduce(
                    tn[:, 0:ga], gat[:, 0:ga], axis=X, op=Alu.add
                )
                nc.vector.copy_predicated(
                    tagc[:, 0:ga], maskp[:, t, 0:ga], tn[:, 0:ga]
                )

            # ---------------- epilogue: mask + int cast + store ----------------
            pm = pp.tile([P, T, G], f32, tag="pm")
            nc.vector.tensor_tensor(pm[:], ph[:], mask[:], op=Alu.mult)
            pi = pp.tile([P, T, G], i32, tag="pi")
            nc.vector.tensor_copy(pi[:], pm[:])
            nc.sync.dma_start(path_d[:], pi[:])
            if debug_outs:
                bpf = pp.tile([P, T, G, K], f32, tag="bpf")
                nc.vector.tensor_copy(bpf[:], bp[:])
                nc.sync.dma_start(bp_dbg[:], bpf[:])
                nc.sync.dma_start(ph_dbg[:], ph[:])

    nc.compile()
    return nc


def get_nc(T, Tc, lg):
    key = (T, Tc, lg)
    if key not in _BUILD_CACHE:
        _BUILD_CACHE[key] = build_nc(T, Tc, lg=lg)
    return _BUILD_CACHE[key]


def core_order_idx(order, c):
    """Sorted position s -> (core c = s % 8, rank r = s//8, g = r//128, p = r%128)."""
    return order[c::NCORES].reshape(G, P)  # [g, p] -> original batch index


def prep_inputs(feats, lengths, transitions, order):
    """Host-side shard/layout prep (length-sorted, striped across cores)."""
    feats = np.asarray(feats, dtype=np.float32)
    lengths = np.asarray(lengths, dtype=np.int32)
    transitions = np.asarray(transitions, dtype=np.float32)
    B, T, Kt = feats.shape
    assert Kt == K and B == NCORES * G * P

    consts = np.zeros((P, 128), dtype=np.float32)
    consts[:, 0:100] = transitions.reshape(-1)[None, :]
    consts[:, 100:110] = (9.0 - np.arange(K, dtype=np.float32))[None, :]
    consts[:, 110:120] = np.arange(K, dtype=np.float32)[None, :]

    in_maps = []
    for c in range(NCORES):
        idx = core_order_idx(order, c)  # [G, P]
        fc = np.ascontiguousarray(
            feats[idx].transpose(1, 2, 0, 3)
        )  # [G,P,T,K] -> [P,T,G,K]
        lc = np.ascontiguousarray(lengths[idx].T).astype(np.float32)  # [P,G]
        in_maps.append({"feats": fc, "lens": lc, "consts": consts})
    return in_maps


def gather_outputs(results, T, order):
    """results: list of per-core {'scores': [P,G], 'path': [P,T,G]}"""
    B = NCORES * G * P
    scores = np.empty((B,), dtype=np.float32)
    path = np.empty((B, T), dtype=np.int32)
    for c, r in enumerate(results):
        idx = core_order_idx(order, c).reshape(-1)  # [G*P]
        scores[idx] = r["scores"].T.reshape(-1)
        path[idx] = r["path"].transpose(2, 0, 1).reshape(G * P, T)
    return scores, path


def run_spmd(nc, in_maps):
    """Execute the bass program on the 8 cores via the axon PJRT path."""
    from concourse import bass2jax

    results = bass2jax.run_bass_via_pjrt(nc, in_maps, n_cores=NCORES)
    return results


def kernel(feats, lengths, transitions):
    feats = np.asarray(feats, dtype=np.float32)
    lengths = np.asarray(lengths, dtype=np.int32)
    B, T, Kt = feats.shape
    Tc = 70 if T % 70 == 0 else T

    order = np.argsort(-lengths, kind="stable")
    sorted_lens = lengths[order]
    lg = tuple(int(sorted_lens[g * NCORES * P]) for g in range(G))

    nc = get_nc(T, Tc, lg)
    in_maps = prep_inputs(feats, lengths, transitions, order)
    results = run_spmd(nc, in_maps)
    scores, path = gather_outputs(results, T, order)
    return scores, path


# revision 25
# speedup vs baseline: 2962.9617x; 2962.9617x over previous
"""Batched CNN-CRF Viterbi decode on 8 Trainium2 NeuronCores.

Problem: feats [4096, 700, 10] f32, lengths [4096] i32, transitions [10,10] f32.
Returns (path_scores [4096] f32, best_paths [4096, 700] i32) exactly matching
the jax reference (sequential Viterbi scan + backtrace, masked past lengths).

Sharding: pure data parallel. Batch 4096 -> 8 cores x 512 sequences.
Per-core layout: 512 = 4 groups x 128 partitions; tags live in the free dim.
Batches are length-sorted and striped across cores so every core sees the
same length profile; group g only needs lg[g] time steps, and every per-step
instruction covers only the still-active group prefix.

Exact structural reduction ("K8"): the CRF fixes trans[START,:] = -1e4 and
trans[:,STOP] = -1e4 while every other entry and all feats are O(1) (margin
~1e4 vs value spreads of O(30)). Hence for t >= 1 the argmax over prev can
never be START or STOP, the forward values of next in {START, STOP} are dead,
and at t = 0 the argmax is START for every (b, next) while bp_0 is never read
by the backtrace. So the scan runs on an 8x8 tag block, with fv_1 computed
analytically as trans[n, START] + feat_0[n].

Per forward step t>=1 (all 512 sequences per instruction, FD=(g,next,prev)):
  ntv  = fv[p,g,1,prev] + trans[p,1,next,prev]          (DVE tensor_tensor add)
  vv   = reduce_max(ntv, axis=prev)                     (DVE tensor_reduce)
  eq   = ntv >= vv                                      (DVE is_ge)
  sel  = eq * (9 - prev)                                (GPSIMD mult, off critical path)
  bprv = reduce_max(sel, axis=prev)  -> 9 - argmax_first(ntv)  (DVE)
  bp[t]= int8(9 - bprv)  (true argmax, first-match ties like jnp.argmax)  (ACT)
  fvn  = vv + feat[t]                                   (DVE)
  fv   = where(t < len, fvn, fv)                        (DVE copy_predicated)
Backpointers stay in SBUF (int8); backtrace runs on-chip via a one-hot
multiply + segmented sum gather; frozen lanes handled with the same mask so
bp past length never has to be identity.
"""

import os
import sys
import numpy as np

sys.path.insert(0, "/opt/trn_rl_repo")
sys.path.insert(0, "/root/.axon_site/_ro/pypackages")

P = 128          # SBUF partitions
G = 4            # batch groups per core (G*P = 512 sequences/core)
K = 10           # tagset size
KR = 8           # restricted tag block (excludes START/STOP)
NCORES = 8
START_TAG = 8
STOP_TAG = 9
NEG = -10000.0

_BUILD_CACHE = {}


def build_nc(T, Tc, kr=KR, debug_outs=False, lg=None, eq_eng="vector", sel_eng="gpsimd"):
    """Build the per-core Bass program (SPMD: same program on all cores)."""
    import concourse.mybir as mybir
    from concourse import bacc
    import concourse.tile as tile

    f32 = mybir.dt.float32
    i32 = mybir.dt.int32
    i8 = mybir.dt.int8
    Alu = mybir.AluOpType
    X = mybir.AxisListType.X
    Copy = mybir.ActivationFunctionType.Copy

    if lg is None:
        lg = (T,) * G
    assert len(lg) == G and all(lg[i] >= lg[i + 1] for i in range(G - 1))
    Lmax = int(lg[0])
    assert lg[G - 1] >= 1

    def n_active(t):
        return sum(1 for x in lg if x > t)

    nc = bacc.Bacc(None, target_bir_lowering=False)

    # feats pre-restricted to tags 0..7 on the host
    feats_d = nc.dram_tensor("feats", (P, T, G, kr), f32, kind="ExternalInput")
    lens_d = nc.dram_tensor("lens", (P, G), f32, kind="ExternalInput")
    consts_d = nc.dram_tensor("consts", (P, 128), f32, kind="ExternalInput")
    scores_d = nc.dram_tensor("scores", (P, G), f32, kind="ExternalOutput")
    path_d = nc.dram_tensor("path", (P, T, G), i32, kind="ExternalOutput")
    if debug_outs:
        bp_dbg = nc.dram_tensor("bp_dbg", (P, T, G, kr), f32, kind="ExternalOutput")
        ph_dbg = nc.dram_tensor("ph_dbg", (P, T, G), f32, kind="ExternalOutput")

    with tile.TileContext(nc) as tc:
        with (
            tc.tile_pool(name="persist", bufs=1) as pp,
            tc.tile_pool(name="fstream", bufs=2) as fp,
            tc.tile_pool(name="work", bufs=3) as wp,
        ):
            consts = pp.tile([P, 128], f32, tag="consts")
            nc.sync.dma_start(consts[:], consts_d[:])
            lens = pp.tile([P, G], f32, tag="lens")
            nc.sync.dma_start(lens[:], lens_d[:])

            fv = pp.tile([P, G, kr], f32, tag="fv")
            bp = pp.tile([P, T, G, kr], i8, tag="bp")
            ph = pp.tile([P, T, G], f32, tag="ph")
            mask = pp.tile([P, T, G], f32, tag="mask")
            maskp = pp.tile([P, T, G], i8, tag="maskp")
            maskx = pp.tile([P, T, G, kr], i8, tag="maskx")
            iota_t = pp.tile([P, T], i32, tag="iota_t")
            iota_f = pp.tile([P, T], f32, tag="iota_f")
            tagc = pp.tile([P, G], f32, tag="tagc")
            sc = pp.tile([P, G], f32, tag="sc")

            # ph regions past each group's maxlen are never written by the
            # backtrace; zero once so the final mask-multiply can't hit NaNs.
            nc.vector.memset(ph[:], 0.0)

            # mask[p, t, g] = (t < len[p, g])
            nc.gpsimd.iota(iota_t[:], pattern=[[1, T]], base=0, channel_multiplier=0)
            nc.vector.tensor_copy(iota_f[:], iota_t[:])
            for g in range(G):
                nc.vector.tensor_scalar(
                    mask[:, :, g], iota_f[:], lens[:, g : g + 1], None, op0=Alu.is_lt
                )
                nc.vector.tensor_scalar(
                    maskp[:, :, g], iota_f[:], lens[:, g : g + 1], None, op0=Alu.is_lt
                )
                nc.vector.tensor_scalar(
                    maskx[:, :, g, :],
                    iota_f[:].unsqueeze(2).broadcast_to((P, T, kr)),
                    lens[:, g : g + 1],
                    None,
                    op0=Alu.is_lt,
                )

            # constant views (consts columns):
            #   0:100   transitions (next-major 10x10)
            #   100:110 reversed iota 9..0
            #   110:120 iota 0..9
            trans10 = consts[:, 0:100].rearrange("p (n k) -> p n k", n=K)
            trans_b = {
                ga: trans10[:, 0:kr, 0:kr].unsqueeze(1).broadcast_to((P, ga, kr, kr))
                for ga in range(1, G + 1)
            }
            # trans[n, START] column for n in 0..7 (t=0 analytic step)
            tstartcol_b = {
                ga: trans10[:, 0:kr, START_TAG]
                .unsqueeze(1)
                .broadcast_to((P, ga, kr))
                for ga in range(1, G + 1)
            }
            rev_b4 = {
                ga: consts[:, 100 : 100 + kr]
                .unsqueeze(1)
                .unsqueeze(1)
                .broadcast_to((P, ga, kr, kr))
                for ga in range(1, G + 1)
            }
            rev_b3 = consts[:, 100 : 100 + kr].unsqueeze(1).broadcast_to((P, G, kr))
            iota_b3 = {
                ga: consts[:, 110 : 110 + kr].unsqueeze(1).broadcast_to((P, ga, kr))
                for ga in range(1, G + 1)
            }
            # trans[STOP, n] for n in 0..7
            stop_b = trans10[:, STOP_TAG, 0:kr].unsqueeze(1).broadcast_to((P, G, kr))

            # ---------------- forward scan ----------------
            nchunks_f = (Lmax + Tc - 1) // Tc
            for c in range(nchunks_f):
                t0c = c * Tc
                tc_len = min(Tc, Lmax - t0c)
                # always DMA all G groups: keeps per-partition runs contiguous
                # (a group-slice would shatter the transfer into 32-96B runs)
                ft = fp.tile([P, Tc, G, kr], f32, tag="ft")
                nc.sync.dma_start(
                    ft[:, 0:tc_len, :, :],
                    feats_d[:, t0c : t0c + tc_len, :, :],
                )
                for tl in range(tc_len):
                    t = t0c + tl
                    ga = n_active(t)
                    if t == 0:
                        # analytic first step: fv1 = trans[n, START] + feat_0
                        # (len >= 1 always, so no freeze; bp_0 never read)
                        nc.vector.tensor_tensor(
                            fv[:, 0:ga],
                            tstartcol_b[ga],
                            ft[:, 0, 0:ga, :],
                            op=Alu.add,
                        )
                        continue
                    ntv = wp.tile([P, G, kr, kr], f32, tag="ntv")
                    fv_b = fv[:, 0:ga].unsqueeze(2).broadcast_to((P, ga, kr, kr))
                    nc.vector.tensor_tensor(
                        ntv[:, 0:ga], fv_b, trans_b[ga], op=Alu.add
                    )

                    vv = wp.tile([P, G, kr], f32, tag="vv")
                    nc.vector.tensor_reduce(
                        vv[:, 0:ga], ntv[:, 0:ga], axis=X, op=Alu.max
                    )

                    eq = wp.tile([P, G, kr, kr], f32, tag="eq")
                    vv_b = vv[:, 0:ga].unsqueeze(3).broadcast_to((P, ga, kr, kr))
                    getattr(nc, eq_eng).tensor_tensor(
                        eq[:, 0:ga], ntv[:, 0:ga], vv_b, op=Alu.is_ge
                    )

                    sel = wp.tile([P, G, kr, kr], f32, tag="sel")
                    getattr(nc, sel_eng).tensor_tensor(
                        sel[:, 0:ga], eq[:, 0:ga], rev_b4[ga], op=Alu.mult
                    )

                    bpr = wp.tile([P, G, kr], f32, tag="bpr")
                    nc.vector.tensor_reduce(
                        bpr[:, 0:ga], sel[:, 0:ga], axis=X, op=Alu.max
                    )

                    # bp[t] = 9 - bpr (true first-match argmax), cast to int8
                    nc.scalar.activation(
                        bp[:, t, 0:ga, :], bpr[:, 0:ga], Copy, bias=9.0, scale=-1.0
                    )

                    fvn = wp.tile([P, G, kr], f32, tag="fvn")
                    nc.vector.tensor_tensor(
                        fvn[:, 0:ga], vv[:, 0:ga], ft[:, tl, 0:ga, :], op=Alu.add
                    )
                    nc.vector.copy_predicated(
                        fv[:, 0:ga], maskx[:, t, 0:ga, :], fvn[:, 0:ga]
                    )

            # ---------------- terminal ----------------
            term = wp.tile([P, G, kr], f32, tag="fvn")
            nc.vector.tensor_tensor(term[:], fv[:], stop_b, op=Alu.add)
            nc.vector.tensor_reduce(sc[:], term[:], axis=X, op=Alu.max)
            nc.sync.dma_start(scores_d[:], sc[:])

            eqt = wp.tile([P, G, kr], f32, tag="vv")
            sc_b = sc[:].unsqueeze(2).broadcast_to((P, G, kr))
            nc.vector.tensor_tensor(eqt[:], term[:], sc_b, op=Alu.is_ge)
            selt = wp.tile([P, G, kr], f32, tag="bpr")
            nc.vector.tensor_tensor(selt[:], eqt[:], rev_b3, op=Alu.mult)
            ltr = wp.tile([P, G], f32, tag="ltr")
            nc.vector.tensor_reduce(ltr[:], selt[:], axis=X, op=Alu.max)
            nc.scalar.activation(tagc[:], ltr[:], Copy, bias=9.0, scale=-1.0)

            # ---------------- backtrace ----------------
            for t in reversed(range(Lmax)):
                ga = n_active(t)
                # path[t] = current tag (ACT engine, off the DVE chain)
                nc.scalar.activation(ph[:, t, 0:ga], tagc[:, 0:ga], Copy)
                if t == 0:
                    break  # bp_0's gather result is discarded by the scan
                oh = wp.tile([P, G, kr], f32, tag="oh")
                tag_b = tagc[:, 0:ga].unsqueeze(2).broadcast_to((P, ga, kr))
                nc.vector.tensor_tensor(
                    oh[:, 0:ga], iota_b3[ga], tag_b, op=Alu.is_equal
                )
                gat = wp.tile([P, G, kr], f32, tag="gat")
                nc.vector.tensor_tensor(
                    gat[:, 0:ga], oh[:, 0:ga], bp[:, t, 0:ga, :], op=Alu.mult
                )
                tn = wp.tile([P, G], f32, tag="tn")
                nc.vector.tensor_reduce(
                    tn[:, 0:ga], gat[:, 0:ga], axis=X, op=Alu.add
                )
                nc.vector.copy_predicated(
                    tagc[:, 0:ga], maskp[:, t, 0:ga], tn[:, 0:ga]
                )

            # ---------------- epilogue: mask + int cast + store ----------------
            pm = pp.tile([P, T, G], f32, tag="pm")
            nc.vector.tensor_tensor(pm[:], ph[:], mask[:], op=Alu.mult)
            pi = pp.tile([P, T, G], i32, tag="pi")
            nc.vector.tensor_copy(pi[:], pm[:])
            nc.sync.dma_start(path_d[:], pi[:])
            if debug_outs:
                bpf = pp.tile([P, T, G, kr], f32, tag="bpf")
                nc.vector.tensor_copy(bpf[:], bp[:])
                nc.sync.dma_start(bp_dbg[:], bpf[:])
                nc.sync.dma_start(ph_dbg[:], ph[:])

    nc.compile()
    return nc


def build_null_nc(T, kr=KR):
    """Same I/O signature as build_nc, near-zero work (timing baseline)."""
    import concourse.mybir as mybir
    from concourse import bacc
    import concourse.tile as tile

    f32 = mybir.dt.float32
    i32 = mybir.dt.int32
    nc = bacc.Bacc(None, target_bir_lowering=False)
    nc.dram_tensor("feats", (P, T, G, kr), f32, kind="ExternalInput")
    nc.dram_tensor("lens", (P, G), f32, kind="ExternalInput")
    consts_d = nc.dram_tensor("consts", (P, 128), f32, kind="ExternalInput")
    scores_d = nc.dram_tensor("scores", (P, G), f32, kind="ExternalOutput")
    path_d = nc.dram_tensor("path", (P, T, G), i32, kind="ExternalOutput")
    with tile.TileContext(nc) as tc:
        with tc.tile_pool(name="p", bufs=1) as pp:
            t = pp.tile([P, G], f32, tag="t")
            nc.sync.dma_start(t[:], consts_d[:, 0:G])
            nc.sync.dma_start(scores_d[:], t[:])
            ti = pp.tile([P, G], i32, tag="ti")
            nc.vector.tensor_copy(ti[:], t[:])
            nc.sync.dma_start(path_d[:, 0, :], ti[:])
    nc.compile()
    return nc


def get_nc(T, Tc, lg, kr):
    key = (T, Tc, lg, kr)
    if key not in _BUILD_CACHE:
        _BUILD_CACHE[key] = build_nc(T, Tc, kr=kr, lg=lg)
    return _BUILD_CACHE[key]


def core_order_idx(order, c):
    """Sorted position s -> (core c = s % 8, rank r = s//8, g = r//128, p = r%128)."""
    return order[c::NCORES].reshape(G, P)  # [g, p] -> original batch index


def prep_inputs(feats, lengths, transitions, order, kr=KR):
    """Host-side shard/layout prep (length-sorted, striped across cores)."""
    feats = np.asarray(feats, dtype=np.float32)
    lengths = np.asarray(lengths, dtype=np.int32)
    transitions = np.asarray(transitions, dtype=np.float32)
    B, T, Kt = feats.shape
    assert Kt == K and B == NCORES * G * P

    consts = np.zeros((P, 128), dtype=np.float32)
    consts[:, 0:100] = transitions.reshape(-1)[None, :]
    consts[:, 100:110] = (9.0 - np.arange(K, dtype=np.float32))[None, :]
    consts[:, 110:120] = np.arange(K, dtype=np.float32)[None, :]

    in_maps = []
    for c in range(NCORES):
        idx = core_order_idx(order, c)  # [G, P]
        fc = np.ascontiguousarray(
            feats[idx, :, 0:kr].transpose(1, 2, 0, 3)
        )  # [G,P,T,kr] -> [P,T,G,kr]
        lc = np.ascontiguousarray(lengths[idx].T).astype(np.float32)  # [P,G]
        in_maps.append({"feats": fc, "lens": lc, "consts": consts})
    return in_maps


def gather_outputs(results, T, order):
    """results: list of per-core {'scores': [P,G], 'path': [P,T,G]}"""
    B = NCORES * G * P
    scores = np.empty((B,), dtype=np.float32)
    path = np.empty((B, T), dtype=np.int32)
    for c, r in enumerate(results):
        idx = core_order_idx(order, c).reshape(-1)  # [G*P]
        scores[idx] = np.asarray(r["scores"]).T.reshape(-1)
        path[idx] = np.asarray(r["path"]).transpose(2, 0, 1).reshape(G * P, T)
    return scores, path


def run_spmd(nc, in_maps):
    """Execute the bass program on the 8 cores via the axon PJRT path."""
    from concourse import bass2jax

    return bass2jax.run_bass_via_pjrt(nc, in_maps, n_cores=NCORES)


def _k8_safe(feats, transitions):
    """Margin check for the 8x8 restriction: START/STOP rows/cols must be
    exactly NEG, and regular magnitudes must be far below |NEG|/2."""
    tr = np.asarray(transitions, dtype=np.float32)
    if not (np.all(tr[START_TAG, :] == NEG) and np.all(tr[:, STOP_TAG] == NEG)):
        return False
    reg = np.abs(np.delete(np.delete(tr, START_TAG, 0), STOP_TAG, 1))
    fmax = float(np.abs(feats).max())
    return float(reg.max()) + fmax < -NEG / 4


def kernel(feats, lengths, transitions):
    feats = np.asarray(feats, dtype=np.float32)
    lengths = np.asarray(lengths, dtype=np.int32)
    B, T, Kt = feats.shape
    Tc = 70 if T % 70 == 0 else T

    order = np.argsort(-lengths, kind="stable")
    sorted_lens = lengths[order]
    lg = tuple(int(sorted_lens[g * NCORES * P]) for g in range(G))

    kr = KR if _k8_safe(feats, transitions) else K
    nc = get_nc(T, Tc, lg, kr)
    in_maps = prep_inputs(feats, lengths, transitions, order, kr)
    results = run_spmd(nc, in_maps)
    scores, path = gather_outputs(results, T, order)
    return scores, path


# revision 27
# speedup vs baseline: 3785.8952x; 1.2777x over previous
"""Batched CNN-CRF Viterbi decode on 8 Trainium2 NeuronCores.

Problem: feats [4096, 700, 10] f32, lengths [4096] i32, transitions [10,10] f32.
Returns (path_scores [4096] f32, best_paths [4096, 700] i32) exactly matching
the jax reference (sequential Viterbi scan + backtrace, masked past lengths).

Sharding: pure data parallel. Batch 4096 -> 8 cores x 512 sequences.
Per-core layout: 512 = 4 groups x 128 partitions; tags live in the free dim.
Batches are length-sorted and striped across cores so every core sees the
same length profile; group g only needs lg[g] time steps, and every per-step
instruction covers only the still-active group prefix.

Exact structural reduction ("K8"): the CRF fixes trans[START,:] = -1e4 and
trans[:,STOP] = -1e4 while every other entry and all feats are O(1) (margin
~1e4 vs value spreads of O(30)). Hence for t >= 1 the argmax over prev can
never be START or STOP, the forward values of next in {START, STOP} are dead,
and at t = 0 the argmax is START for every (b, next) while bp_0 is never read
by the backtrace. So the scan runs on an 8x8 tag block, with fv_1 computed
analytically as trans[n, START] + feat_0[n].

Per forward step t>=1 (all 512 sequences per instruction, FD=(g,next,prev)):
  ntv  = fv[p,g,1,prev] + trans[p,1,next,prev]          (DVE tensor_tensor add)
  vv   = reduce_max(ntv, axis=prev)                     (DVE tensor_reduce)
  eq   = ntv >= vv                                      (DVE is_ge)
  sel  = eq * (9 - prev)                                (GPSIMD mult, off critical path)
  bprv = reduce_max(sel, axis=prev)  -> 9 - argmax_first(ntv)  (DVE)
  bp[t]= int8(9 - bprv)  (true argmax, first-match ties like jnp.argmax)  (ACT)
  fvn  = vv + feat[t]                                   (DVE)
  fv   = where(t < len, fvn, fv)                        (DVE copy_predicated)
Backpointers stay in SBUF (int8); backtrace runs on-chip via a one-hot
multiply + segmented sum gather; frozen lanes handled with the same mask so
bp past length never has to be identity.
"""

import os
import sys
import numpy as np

sys.path.insert(0, "/opt/trn_rl_repo")
sys.path.insert(0, "/root/.axon_site/_ro/pypackages")

P = 128          # SBUF partitions
G = 4            # batch groups per core (G*P = 512 sequences/core)
K = 10           # tagset size
KR = 8           # restricted tag block (excludes START/STOP)
NCORES = 8
START_TAG = 8
STOP_TAG = 9
NEG = -10000.0

_BUILD_CACHE = {}


def build_nc(T, Tc, kr=KR, debug_outs=False, lg=None, eq_eng="vector", sel_eng="vector"):
    """Build the per-core Bass program (SPMD: same program on all cores)."""
    import concourse.mybir as mybir
    from concourse import bacc
    import concourse.tile as tile

    f32 = mybir.dt.float32
    i32 = mybir.dt.int32
    i8 = mybir.dt.int8
    Alu = mybir.AluOpType
    X = mybir.AxisListType.X
    Copy = mybir.ActivationFunctionType.Copy

    if lg is None:
        lg = (T,) * G
    assert len(lg) == G and all(lg[i] >= lg[i + 1] for i in range(G - 1))
    Lmax = int(lg[0])
    assert lg[G - 1] >= 1

    def n_active(t):
        return sum(1 for x in lg if x > t)

    nc = bacc.Bacc(None, target_bir_lowering=False)

    # feats pre-restricted to tags 0..7 on the host
    feats_d = nc.dram_tensor("feats", (P, T, G, kr), f32, kind="ExternalInput")
    lens_d = nc.dram_tensor("lens", (P, G), f32, kind="ExternalInput")
    consts_d = nc.dram_tensor("consts", (P, 128), f32, kind="ExternalInput")
    scores_d = nc.dram_tensor("scores", (P, G), f32, kind="ExternalOutput")
    path_d = nc.dram_tensor("path", (P, T, G), i32, kind="ExternalOutput")
    if debug_outs:
        bp_dbg = nc.dram_tensor("bp_dbg", (P, T, G, kr), f32, kind="ExternalOutput")
        ph_dbg = nc.dram_tensor("ph_dbg", (P, T, G), f32, kind="ExternalOutput")

    with tile.TileContext(nc) as tc:
        with (
            tc.tile_pool(name="persist", bufs=1) as pp,
            tc.tile_pool(name="fstream", bufs=2) as fp,
            tc.tile_pool(name="work", bufs=3) as wp,
        ):
            consts = pp.tile([P, 128], f32, tag="consts")
            nc.sync.dma_start(consts[:], consts_d[:])
            lens = pp.tile([P, G], f32, tag="lens")
            nc.sync.dma_start(lens[:], lens_d[:])

            fv = pp.tile([P, G, kr], f32, tag="fv")
            bp = pp.tile([P, T, G, kr], i8, tag="bp")
            ph = pp.tile([P, T, G], f32, tag="ph")
            mask = pp.tile([P, T, G], f32, tag="mask")
            maskp = pp.tile([P, T, G], i8, tag="maskp")
            maskx = pp.tile([P, T, G, kr], i8, tag="maskx")
            iota_t = pp.tile([P, T], i32, tag="iota_t")
            iota_f = pp.tile([P, T], f32, tag="iota_f")
            tagc = pp.tile([P, G], f32, tag="tagc")
            sc = pp.tile([P, G], f32, tag="sc")

            # ph regions past each group's maxlen are never written by the
            # backtrace; zero once so the final mask-multiply can't hit NaNs.
            nc.vector.memset(ph[:], 0.0)

            # mask[p, t, g] = (t < len[p, g])
            nc.gpsimd.iota(iota_t[:], pattern=[[1, T]], base=0, channel_multiplier=0)
            nc.vector.tensor_copy(iota_f[:], iota_t[:])
            for g in range(G):
                nc.vector.tensor_scalar(
                    mask[:, :, g], iota_f[:], lens[:, g : g + 1], None, op0=Alu.is_lt
                )
                nc.vector.tensor_scalar(
                    maskp[:, :, g], iota_f[:], lens[:, g : g + 1], None, op0=Alu.is_lt
                )
                nc.vector.tensor_scalar(
                    maskx[:, :, g, :],
                    iota_f[:].unsqueeze(2).broadcast_to((P, T, kr)),
                    lens[:, g : g + 1],
                    None,
                    op0=Alu.is_lt,
                )

            # constant views (consts columns):
            #   0:100   transitions (next-major 10x10)
            #   100:110 reversed iota 9..0
            #   110:120 iota 0..9
            trans10 = consts[:, 0:100].rearrange("p (n k) -> p n k", n=K)
            trans_b = {
                ga: trans10[:, 0:kr, 0:kr].unsqueeze(1).broadcast_to((P, ga, kr, kr))
                for ga in range(1, G + 1)
            }
            # trans[n, START] column for n in 0..7 (t=0 analytic step)
            tstartcol_b = {
                ga: trans10[:, 0:kr, START_TAG]
                .unsqueeze(1)
                .broadcast_to((P, ga, kr))
                for ga in range(1, G + 1)
            }
            rev_b4 = {
                ga: consts[:, 100 : 100 + kr]
                .unsqueeze(1)
                .unsqueeze(1)
                .broadcast_to((P, ga, kr, kr))
                for ga in range(1, G + 1)
            }
            rev_b3 = consts[:, 100 : 100 + kr].unsqueeze(1).broadcast_to((P, G, kr))
            iota_b3 = {
                ga: consts[:, 110 : 110 + kr].unsqueeze(1).broadcast_to((P, ga, kr))
                for ga in range(1, G + 1)
            }
            # trans[STOP, n] for n in 0..7
            stop_b = trans10[:, STOP_TAG, 0:kr].unsqueeze(1).broadcast_to((P, G, kr))

            # ---------------- forward scan ----------------
            nchunks_f = (Lmax + Tc - 1) // Tc
            for c in range(nchunks_f):
                t0c = c * Tc
                tc_len = min(Tc, Lmax - t0c)
                # always DMA all G groups: keeps per-partition runs contiguous
                # (a group-slice would shatter the transfer into 32-96B runs)
                ft = fp.tile([P, Tc, G, kr], f32, tag="ft")
                nc.sync.dma_start(
                    ft[:, 0:tc_len, :, :],
                    feats_d[:, t0c : t0c + tc_len, :, :],
                )
                for tl in range(tc_len):
                    t = t0c + tl
                    ga = n_active(t)
                    if t == 0:
                        # analytic first step: fv1 = trans[n, START] + feat_0
                        # (len >= 1 always, so no freeze; bp_0 never read)
                        nc.vector.tensor_tensor(
                            fv[:, 0:ga],
                            tstartcol_b[ga],
                            ft[:, 0, 0:ga, :],
                            op=Alu.add,
                        )
                        continue
                    ntv = wp.tile([P, G, kr, kr], f32, tag="ntv")
                    fv_b = fv[:, 0:ga].unsqueeze(2).broadcast_to((P, ga, kr, kr))
                    nc.vector.tensor_tensor(
                        ntv[:, 0:ga], fv_b, trans_b[ga], op=Alu.add
                    )

                    vv = wp.tile([P, G, kr], f32, tag="vv")
                    nc.vector.tensor_reduce(
                        vv[:, 0:ga], ntv[:, 0:ga], axis=X, op=Alu.max
                    )

                    eq = wp.tile([P, G, kr, kr], f32, tag="eq")
                    vv_b = vv[:, 0:ga].unsqueeze(3).broadcast_to((P, ga, kr, kr))
                    getattr(nc, eq_eng).tensor_tensor(
                        eq[:, 0:ga], ntv[:, 0:ga], vv_b, op=Alu.is_ge
                    )

                    sel = wp.tile([P, G, kr, kr], f32, tag="sel")
                    getattr(nc, sel_eng).tensor_tensor(
                        sel[:, 0:ga], eq[:, 0:ga], rev_b4[ga], op=Alu.mult
                    )

                    bpr = wp.tile([P, G, kr], f32, tag="bpr")
                    nc.vector.tensor_reduce(
                        bpr[:, 0:ga], sel[:, 0:ga], axis=X, op=Alu.max
                    )

                    # bp[t] = 9 - bpr (true first-match argmax), cast to int8
                    nc.scalar.activation(
                        bp[:, t, 0:ga, :], bpr[:, 0:ga], Copy, bias=9.0, scale=-1.0
                    )

                    fvn = wp.tile([P, G, kr], f32, tag="fvn")
                    nc.vector.tensor_tensor(
                        fvn[:, 0:ga], vv[:, 0:ga], ft[:, tl, 0:ga, :], op=Alu.add
                    )
                    nc.vector.copy_predicated(
                        fv[:, 0:ga], maskx[:, t, 0:ga, :], fvn[:, 0:ga]
                    )

            # ---------------- terminal ----------------
            term = wp.tile([P, G, kr], f32, tag="fvn")
            nc.vector.tensor_tensor(term[:], fv[:], stop_b, op=Alu.add)
            nc.vector.tensor_reduce(sc[:], term[:], axis=X, op=Alu.max)
            nc.sync.dma_start(scores_d[:], sc[:])

            eqt = wp.tile([P, G, kr], f32, tag="vv")
            sc_b = sc[:].unsqueeze(2).broadcast_to((P, G, kr))
            nc.vector.tensor_tensor(eqt[:], term[:], sc_b, op=Alu.is_ge)
            selt = wp.tile([P, G, kr], f32, tag="bpr")
            nc.vector.tensor_tensor(selt[:], eqt[:], rev_b3, op=Alu.mult)
            ltr = wp.tile([P, G], f32, tag="ltr")
            nc.vector.tensor_reduce(ltr[:], selt[:], axis=X, op=Alu.max)
            nc.scalar.activation(tagc[:], ltr[:], Copy, bias=9.0, scale=-1.0)

            # ---------------- backtrace ----------------
            for t in reversed(range(Lmax)):
                ga = n_active(t)
                # path[t] = current tag (ACT engine, off the DVE chain)
                nc.scalar.activation(ph[:, t, 0:ga], tagc[:, 0:ga], Copy)
                if t == 0:
                    break  # bp_0's gather result is discarded by the scan
                oh = wp.tile([P, G, kr], f32, tag="oh")
                tag_b = tagc[:, 0:ga].unsqueeze(2).broadcast_to((P, ga, kr))
                nc.vector.tensor_tensor(
                    oh[:, 0:ga], iota_b3[ga], tag_b, op=Alu.is_equal
                )
                gat = wp.tile([P, G, kr], f32, tag="gat")
                nc.vector.tensor_tensor(
                    gat[:, 0:ga], oh[:, 0:ga], bp[:, t, 0:ga, :], op=Alu.mult
                )
                tn = wp.tile([P, G], f32, tag="tn")
                nc.vector.tensor_reduce(
                    tn[:, 0:ga], gat[:, 0:ga], axis=X, op=Alu.add
                )
                nc.vector.copy_predicated(
                    tagc[:, 0:ga], maskp[:, t, 0:ga], tn[:, 0:ga]
                )

            # ---------------- epilogue: mask + int cast + store ----------------
            pm = pp.tile([P, T, G], f32, tag="pm")
            nc.vector.tensor_tensor(pm[:], ph[:], mask[:], op=Alu.mult)
            pi = pp.tile([P, T, G], i32, tag="pi")
            nc.vector.tensor_copy(pi[:], pm[:])
            nc.sync.dma_start(path_d[:], pi[:])
            if debug_outs:
                bpf = pp.tile([P, T, G, kr], f32, tag="bpf")
                nc.vector.tensor_copy(bpf[:], bp[:])
                nc.sync.dma_start(bp_dbg[:], bpf[:])
                nc.sync.dma_start(ph_dbg[:], ph[:])

    nc.compile()
    return nc


def build_null_nc(T, kr=KR):
    """Same I/O signature as build_nc, near-zero work (timing baseline)."""
    import concourse.mybir as mybir
    from concourse import bacc
    import concourse.tile as tile

    f32 = mybir.dt.float32
    i32 = mybir.dt.int32
    nc = bacc.Bacc(None, target_bir_lowering=False)
    nc.dram_tensor("feats", (P, T, G, kr), f32, kind="ExternalInput")
    nc.dram_tensor("lens", (P, G), f32, kind="ExternalInput")
    consts_d = nc.dram_tensor("consts", (P, 128), f32, kind="ExternalInput")
    scores_d = nc.dram_tensor("scores", (P, G), f32, kind="ExternalOutput")
    path_d = nc.dram_tensor("path", (P, T, G), i32, kind="ExternalOutput")
    with tile.TileContext(nc) as tc:
        with tc.tile_pool(name="p", bufs=1) as pp:
            t = pp.tile([P, G], f32, tag="t")
            nc.sync.dma_start(t[:], consts_d[:, 0:G])
            nc.sync.dma_start(scores_d[:], t[:])
            ti = pp.tile([P, G], i32, tag="ti")
            nc.vector.tensor_copy(ti[:], t[:])
            nc.sync.dma_start(path_d[:, 0, :], ti[:])
    nc.compile()
    return nc


def get_nc(T, Tc, lg, kr):
    key = (T, Tc, lg, kr)
    if key not in _BUILD_CACHE:
        _BUILD_CACHE[key] = build_nc(T, Tc, kr=kr, lg=lg)
    return _BUILD_CACHE[key]


def core_order_idx(order, c):
    """Sorted position s -> (core c = s % 8, rank r = s//8, g = r//128, p = r%128)."""
    return order[c::NCORES].reshape(G, P)  # [g, p] -> original batch index


def prep_inputs(feats, lengths, transitions, order, kr=KR):
    """Host-side shard/layout prep (length-sorted, striped across cores)."""
    feats = np.asarray(feats, dtype=np.float32)
    lengths = np.asarray(lengths, dtype=np.int32)
    transitions = np.asarray(transitions, dtype=np.float32)
    B, T, Kt = feats.shape
    assert Kt == K and B == NCORES * G * P

    consts = np.zeros((P, 128), dtype=np.float32)
    consts[:, 0:100] = transitions.reshape(-1)[None, :]
    consts[:, 100:110] = (9.0 - np.arange(K, dtype=np.float32))[None, :]
    consts[:, 110:120] = np.arange(K, dtype=np.float32)[None, :]

    in_maps = []
    for c in range(NCORES):
        idx = core_order_idx(order, c)  # [G, P]
        fc = np.ascontiguousarray(
            feats[idx, :, 0:kr].transpose(1, 2, 0, 3)
        )  # [G,P,T,kr] -> [P,T,G,kr]
        lc = np.ascontiguousarray(lengths[idx].T).astype(np.float32)  # [P,G]
        in_maps.append({"feats": fc, "lens": lc, "consts": consts})
    return in_maps


def gather_outputs(results, T, order):
    """results: list of per-core {'scores': [P,G], 'path': [P,T,G]}"""
    B = NCORES * G * P
    scores = np.empty((B,), dtype=np.float32)
    path = np.empty((B, T), dtype=np.int32)
    for c, r in enumerate(results):
        idx = core_order_idx(order, c).reshape(-1)  # [G*P]
        scores[idx] = np.asarray(r["scores"]).T.reshape(-1)
        path[idx] = np.asarray(r["path"]).transpose(2, 0, 1).reshape(G * P, T)
    return scores, path


_RUNNER_CACHE = {}


def _make_runner(nc):
    """Compile a reusable jitted SPMD executor for this bass program."""
    import jax
    from jax.sharding import Mesh, PartitionSpec
    from jax.experimental.shard_map import shard_map
    from concourse import bass2jax, mybir

    bass2jax.install_neuronx_cc_hook()
    partition_name = (
        nc.partition_id_tensor.name if nc.partition_id_tensor else None
    )
    in_names, out_names, out_avals = [], [], []
    for alloc in nc.m.functions[0].allocations:
        if not isinstance(alloc, mybir.MemoryLocationSet):
            continue
        name = alloc.memorylocations[0].name
        if alloc.kind == "ExternalInput":
            if name != partition_name:
                in_names.append(name)
        elif alloc.kind == "ExternalOutput":
            out_names.append(name)
            out_avals.append(
                jax.core.ShapedArray(
                    tuple(alloc.tensor_shape), mybir.dt.np(alloc.dtype)
                )
            )
    all_names = tuple(
        in_names + out_names + ([partition_name] if partition_name else [])
    )

    def _body(*args):
        operands = list(args)
        if partition_name is not None:
            operands.append(bass2jax.partition_id_tensor())
        outs = bass2jax._bass_exec_p.bind(
            *operands,
            out_avals=tuple(out_avals),
            in_names=all_names,
            out_names=tuple(out_names),
            lowering_input_output_aliases=(),
            sim_require_finite=True,
            sim_require_nnan=True,
            nc=nc,
        )
        return tuple(outs)

    devices = jax.devices()[:NCORES]
    mesh = Mesh(np.asarray(devices), ("core",))
    nio = len(in_names) + len(out_names)
    f = jax.jit(
        shard_map(
            _body,
            mesh=mesh,
            in_specs=(PartitionSpec("core"),) * nio,
            out_specs=(PartitionSpec("core"),) * len(out_names),
            check_rep=False,
        )
    )
    # pre-staged zero buffers for the output params (contents never read:
    # the program writes every output element)
    from jax.sharding import NamedSharding

    sh = NamedSharding(mesh, PartitionSpec("core"))
    zeros = [
        jax.device_put(
            np.zeros((a.shape[0] * NCORES,) + tuple(a.shape[1:]), a.dtype), sh
        )
        for a in out_avals
    ]
    return f, in_names, out_names, mesh, sh, zeros


def run_spmd(nc, in_maps):
    """Execute the bass program on the 8 cores (cached jitted runner)."""
    import jax

    key = id(nc)
    if key not in _RUNNER_CACHE:
        _RUNNER_CACHE[key] = _make_runner(nc)
    f, in_names, out_names, mesh, sh, zeros = _RUNNER_CACHE[key]

    staged = [
        jax.device_put(
            np.concatenate([np.asarray(m[name]) for m in in_maps], axis=0), sh
        )
        for name in in_names
    ]
    outs = f(*staged, *zeros)
    outs = [np.asarray(o) for o in outs]
    results = []
    for c in range(NCORES):
        results.append(
            {
                name: o[c * o.shape[0] // NCORES : (c + 1) * o.shape[0] // NCORES]
                for name, o in zip(out_names, outs)
            }
        )
    return results


def _k8_safe(feats, transitions):
    """Margin check for the 8x8 restriction: START/STOP rows/cols must be
    exactly NEG, and regular magnitudes must be far below |NEG|/2."""
    tr = np.asarray(transitions, dtype=np.float32)
    if not (np.all(tr[START_TAG, :] == NEG) and np.all(tr[:, STOP_TAG] == NEG)):
        return False
    reg = np.abs(np.delete(np.delete(tr, START_TAG, 0), STOP_TAG, 1))
    fmax = float(np.abs(feats).max())
    return float(reg.max()) + fmax < -NEG / 4


def kernel(feats, lengths, transitions):
    feats = np.asarray(feats, dtype=np.float32)
    lengths = np.asarray(lengths, dtype=np.int32)
    B, T, Kt = feats.shape
    Tc = 70 if T % 70 == 0 else T

    order = np.argsort(-lengths, kind="stable")
    sorted_lens = lengths[order]
    lg = tuple(int(sorted_lens[g * NCORES * P]) for g in range(G))

    kr = KR if _k8_safe(feats, transitions) else K
    nc = get_nc(T, Tc, lg, kr)
    in_maps = prep_inputs(feats, lengths, transitions, order, kr)
    results = run_spmd(nc, in_maps)
    scores, path = gather_outputs(results, T, order)
    return scores, path
